# revision 62
# baseline (speedup 1.0000x reference)
"""GAT (2-layer, mu/std heads) Trainium2 kernel — 8-core SPMD.

Sharding: nodes partitioned into 8 contiguous ranges (dst-sharding); edges
assigned to the core owning their dst, sorted by (dst-tile, src-half, src).

Layer 1 node table (512B records) is computed by EVERY core for ALL nodes
from the replicated bf16 x input — no AllGather. Layer 2 table (256B
records) carries per-node W_mu/W_std projections + attention alphas, so
only 256B/node crosses the wire (one AllGather) and the L2 edge phase
aggregates 32-wide groups directly.

Edge gather via dma_gather by src row, round-robined over the 4 SWDGE
queues so descriptor generation runs on all 8 Q7 cores. alpha_dst is
fetched on-chip: a transposed one-hot (dst-slot x edge) matmuls a per-tile
alpha_dst table. Scatter-add via one-hot matmul; softmax denominators ride
as extra exp() columns in the same matmul.

L1 record (bf16, 256 elems): [0:2]=as [2:4]=ad [4:68]=xp_h0 [132:196]=xp_h1
L2 record (bf16, 128 elems): [0:4]=(asmu,asstd,admu,adstd) [4:36]=xpmu
                             [68:100]=xpstd
"""
import sys
sys.path.insert(0, '/opt/trn_rl_repo')
import numpy as np
import ml_dtypes

BF = ml_dtypes.bfloat16

# ---------------- problem constants (hardcoded per spec) ----------------
N = 50000
F_IN = 128
HID = 64
H = 2
Z = 32
NEG = 0.2
NCORES = 8
NPC = N // NCORES            # 6250 nodes per core
P = 128
NT = (NPC + P - 1) // P      # 49 dst tiles per core
NPCPAD = NT * P              # 6272
NTG = NT * NCORES            # 392 global tiles
SENTROW = NPCPAD - 1         # per-core sentinel row (alpha = -1e30)
RECW = 256                   # L1 record bf16 elems per node row (512 B)
RECW2 = 128                  # L2 record bf16 elems per node row (256 B)
HALFROWS = (NCORES // 2) * NPCPAD   # 25088 rows per half-table
NB = 32                      # blocks per gather batch
GMAXB = 8                    # max blocks (128 idx each) per dma_gather call
BIG = -1.0e30


# ---------------- host-side prep ----------------
def _prep_edges(edges):
    """Shard + sort by (tile, src-half, src) + pad; build packed index arrays.

    Returns (schedule [NT,2] int, half_flags, per-core dict)."""
    src = np.concatenate([edges[0].astype(np.int64), np.arange(N, dtype=np.int64)])
    dst = np.concatenate([edges[1].astype(np.int64), np.arange(N, dtype=np.int64)])
    core = dst // NPC
    dstl = dst - core * NPC
    tile = dstl >> 7
    # L1 table rows are 8-tile interleaved (tiles 8g..8g+7 stored as rows
    # g*1024 + p*8 + i) so the node sweep writes 8 tiles per DMA at 4KB
    # per partition (line rate); the last group of a 49-tile block is a
    # single tile. L2 table rows are natural. Both tables share the same
    # (tile, half, src) edge order.
    sc = src // NPC
    sl = src - sc * NPC
    st, sp = sl >> 7, sl & 127
    gs = np.where(st < 48, 8, 1)
    src_row = sc * NPCPAD + (st >> 3) * 1024 + sp * gs + (st & 7)  # L1 row
    src_row2 = sc * NPCPAD + sl                                    # L2 row
    half = (src_row >= HALFROWS).astype(np.int64)

    counts = np.zeros((NCORES, NT, 2), np.int64)
    np.add.at(counts, (core, tile, half), 1)
    blocks = (counts + P - 1) // P                   # [C, NT, 2]
    schedule = blocks.max(axis=0)                    # [NT, 2]
    schedule[:, 0] = np.maximum(schedule[:, 0], 1)   # >=1 block per tile
    nblk = int(schedule.sum())

    # flat block index of each (tile, half) group start
    grp_blocks = schedule.reshape(-1)                # [NT*2]
    grp_start = np.zeros(NT * 2, np.int64)
    grp_start[1:] = np.cumsum(grp_blocks)[:-1]
    grp_start = grp_start.reshape(NT, 2)

    half_flags = np.zeros(nblk, np.int64)
    for t in range(NT):
        half_flags[grp_start[t, 1]:grp_start[t, 1] + schedule[t, 1]] = 1

    per_core = []
    for c in range(NCORES):
        m = core == c
        key = tile[m] * 2 + half[m]
        srow_m = src_row[m]
        srow2_m = src_row2[m]
        order = np.lexsort((srow_m, key))            # by group, then src asc
        key_s = key[order]
        srow_s = srow_m[order]
        srow2_s = srow2_m[order]
        dstl_s = dstl[m][order]
        half_s = half[m][order]
        cnt = counts[c].reshape(-1)                  # [NT*2]
        g0 = np.zeros(NT * 2, np.int64)
        g0[1:] = np.cumsum(cnt)[:-1]
        k = np.arange(key_s.size, dtype=np.int64) - g0[key_s]
        flat = (grp_start.reshape(-1)[key_s] + (k >> 7)) * P + (k & 127)

        # pad edges read the (harmless) pad row; a -1e30 mask (pmask)
        # zeroes their exp() weight
        esrc = np.full(nblk * P, SENTROW, np.int64)
        esrc2 = np.full(nblk * P, SENTROW, np.int64)
        eslot = np.zeros(nblk * P, np.int64)
        pmask = np.full(nblk * P, -1.0e30, BF)
        esrc[flat] = srow_s - half_s * HALFROWS
        esrc2[flat] = srow2_s - half_s * HALFROWS
        eslot[flat] = dstl_s & 127
        pmask[flat] = 0.0

        # gather-idx wrap-16 packing, replicated to 128 partitions
        def pack16(vals):
            n = vals.size
            t16 = np.zeros((16, n // 16), np.int16)
            t16[np.arange(n) % 16, np.arange(n) // 16] = vals.astype(np.int16)
            return np.ascontiguousarray(np.tile(t16, (8, 1)))

        # esall: [esrc16(L1) | esrc16(L2)] as one int16 tensor
        per_core.append({
            "esall": np.ascontiguousarray(
                np.concatenate([pack16(esrc), pack16(esrc2)], axis=1)),
            "eslot": np.ascontiguousarray(
                eslot.reshape(nblk, P).T.astype(BF)),
            "pmask": np.ascontiguousarray(pmask.reshape(nblk, P).T),
            "eslotf": np.ascontiguousarray(eslot.astype(np.uint8).reshape(1, -1)),
        })
    return schedule, half_flags, per_core


def _prep_weights(W1, a_src1, a_dst1, b1, W_mu, a_src_mu, a_dst_mu, b_mu,
                  W_std, a_src_std, a_dst_std, b_std):
    am1 = np.zeros((F_IN, 4), np.float32)
    am1[0:HID, 0] = a_src1[0]
    am1[HID:2 * HID, 1] = a_src1[1]
    am1[0:HID, 2] = a_dst1[0]
    am1[HID:2 * HID, 3] = a_dst1[1]
    am2 = np.zeros((2 * Z, 4), np.float32)
    am2[0:Z, 0] = a_src_mu[0]
    am2[Z:2 * Z, 1] = a_src_std[0]
    am2[0:Z, 2] = a_dst_mu[0]
    am2[Z:2 * Z, 3] = a_dst_std[0]
    # f32 blob [128, 648]:
    # cols: (unused 0:128) | w1raw 128:256 | amask1 256:260 | wcat 260:388
    #       amask2 388:392 (rows<64) | (unused 392:456)
    #       b1rep 456:584 | b2rep 584:648
    blob = np.zeros((P, 648), np.float32)
    blob[:, 128:256] = W1
    blob[:, 256:260] = am1
    blob[0:2 * Z, 260:388] = np.vstack([W_mu, W_std])
    blob[0:2 * Z, 388:392] = am2
    blob[:, 456:584] = np.tile(b1.astype(np.float32), (P, 1))
    blob[:, 584:616] = np.tile(b_mu.astype(np.float32), (P, 1))
    blob[:, 616:648] = np.tile(b_std.astype(np.float32), (P, 1))
    # bf16 blob [128, 192]: W1.T | W_mu.T | W_std.T
    wbf = np.zeros((P, 192), BF)
    wbf[:, 0:128] = W1.T.astype(BF)
    wbf[:, 128:160] = W_mu.T.astype(BF)
    wbf[:, 160:192] = W_std.T.astype(BF)
    return {"wblob": np.ascontiguousarray(blob),
            "wbf": np.ascontiguousarray(wbf)}


# ---------------- device program ----------------
def _build_nc(schedule, half_flags, sim_single_core=False):
    import concourse.bass as bass
    import concourse.mybir as mybir
    import concourse.tile as tile
    import concourse.bacc as bacc
    from concourse.masks import make_identity

    f32 = mybir.dt.float32
    bf16 = mybir.dt.bfloat16
    fp8 = mybir.dt.float8e4
    i16 = mybir.dt.int16
    AF = mybir.ActivationFunctionType
    OP = mybir.AluOpType
    nblk = int(schedule.sum())

    # flat block index -> dst tile
    tile_of_block = np.zeros(nblk, np.int64)
    b = 0
    for t in range(NT):
        for hf in range(2):
            for _ in range(int(schedule[t, hf])):
                tile_of_block[b] = t
                b += 1

    import os as _os
    no_coll = bool(int(_os.environ.get("GAT_NO_COLL", "0")))

    # The stock cost model charges SWDGE descriptor generation at 0.34
    # ns/desc; dma_gather's Q7 loop measures ~9 ns/desc on HW. The Tile
    # list-scheduler uses this constant, so correct it so gathers are
    # spaced/overlapped realistically in the schedule.
    import concourse.hw_specs as _hw
    _hw.TRN2Spec.SWDGE_NS_PER_DESCRIPTOR = float(
        _os.environ.get("GAT_SWDGE_NS", "9.0"))

    nc = bacc.Bacc("TRN2", target_bir_lowering=False, debug=False,
                   num_devices=(1 if sim_single_core else NCORES),
                   num_swdge_queues=4)

    def _collective(name, op, replica_groups, ins, outs):
        if not sim_single_core and not no_coll:
            nc.gpsimd.collective_compute(name, op, replica_groups=replica_groups,
                                         ins=ins, outs=outs)
            return
        # stub: approximate AllGather receive traffic with local DMA copies
        src, dst = ins[0], outs[0]
        rows = src.shape[0]
        for r in range(NCORES):
            nc.sync.dma_start(out=dst[r * rows:(r + 1) * rows, :], in_=src)

    # ---- I/O (consolidated: per-call axon dispatch overhead scales with
    # the number of buffers, so pack everything into 4 inputs) ----
    XW = NPCPAD * NCORES + NPCPAD + 2 * nblk + 192
    xall_d = nc.dram_tensor("xall", [P, XW], bf16, kind="ExternalInput")
    wblob_d = nc.dram_tensor("wblob", [P, 648], f32, kind="ExternalInput")
    esall_d = nc.dram_tensor("esall", [P, nblk * 16], i16, kind="ExternalInput")
    eslotf_d = nc.dram_tensor("eslotf", [1, nblk * P], mybir.dt.uint8,
                              kind="ExternalInput")
    musd_out = nc.dram_tensor("musd_out", [NPC, 2 * Z], f32, kind="ExternalOutput")
    xfull_d = xall_d[:, 0:NPCPAD * NCORES]
    xown_d = xall_d[:, NPCPAD * NCORES:NPCPAD * NCORES + NPCPAD]
    _eo = NPCPAD * NCORES + NPCPAD
    eslot_d = xall_d[:, _eo:_eo + nblk]
    pmask_d = xall_d[:, _eo + nblk:_eo + 2 * nblk]
    wbf_d = xall_d[:, XW - 192:XW]
    esrc_d = esall_d[:, 0:nblk * 8]
    esrc2_d = esall_d[:, nblk * 8:nblk * 16]

    with tile.TileContext(nc) as tc:
        with tc.tile_pool(name="dram", bufs=1, space="DRAM") as dram, \
             tc.tile_pool(name="const", bufs=1) as cp:
            shared_kw = ({} if (sim_single_core or no_coll)
                         else {"addr_space": "Shared"})
            rec1_full = dram.tile([NPCPAD * NCORES, RECW], bf16)
            rec2_slice = dram.tile([NPCPAD, RECW2], bf16)
            rec2_full = dram.tile([NPCPAD * NCORES, RECW2], bf16, **shared_kw)

            # ---- constants ----
            iota_bf = cp.tile([P, P], bf16)
            nc.gpsimd.iota(iota_bf[:], pattern=[[1, P]], base=0,
                           channel_multiplier=0,
                           allow_small_or_imprecise_dtypes=True)
            iota_col = cp.tile([P, 1], f32)
            nc.gpsimd.iota(iota_col[:], pattern=[[1, 1]], base=0,
                           channel_multiplier=1,
                           allow_small_or_imprecise_dtypes=True)
            ident = cp.tile([P, P], f32)
            make_identity(nc, ident[:])
            # per-tile alpha_dst tables (bf16), cols [ad_h0, ad_h1] per tile
            adtab1 = cp.tile([P, NT * 2], bf16)
            adtab2 = cp.tile([P, NT * 2], bf16)

            wblob_s = cp.tile([P, 648], f32, name="wblob_s")
            nc.sync.dma_start(out=wblob_s[:], in_=wblob_d[:])
            w1raw_s = wblob_s[:, 128:256]
            amask1_s = wblob_s[:, 256:260]
            wcat_s = wblob_s[0:2 * Z, 260:388]
            amask2_s = wblob_s[0:2 * Z, 388:392]
            b1rep_s = wblob_s[:, 456:584]
            b2rep_s = wblob_s[:, 584:648]

            # rhs1 [feat, 132] = [W1.T | u1] (bf16); rhscat [feat, 68] =
            # [u2 | W_mu.T | W_std.T] (bf16)
            rhs1 = cp.tile([P, 132], bf16, name="rhs1")
            rhscat = cp.tile([P, 68], bf16, name="rhscat")
            nc.sync.dma_start(out=rhs1[:, 0:128], in_=wbf_d[:, 0:128])
            nc.sync.dma_start(out=rhscat[:, 4:68], in_=wbf_d[:, 128:192])
            u1bf = cp.tile([F_IN, 4], bf16, name="u1bf")
            with tc.tile_pool(name="ups", bufs=1, space="PSUM") as ups:
                u1_ps = ups.tile([F_IN, 4], f32)
                nc.tensor.matmul(out=u1_ps[:], lhsT=w1raw_s, rhs=amask1_s,
                                 start=True, stop=True)
                nc.vector.tensor_copy(out=u1bf[:], in_=u1_ps[:])
                nc.vector.tensor_copy(out=rhs1[:, 128:132], in_=u1_ps[:])
                u2_ps = ups.tile([F_IN, 4], f32)
                nc.tensor.matmul(out=u2_ps[:], lhsT=wcat_s,
                                 rhs=amask2_s, start=True, stop=True)
                nc.vector.tensor_copy(out=rhscat[:, 0:4], in_=u2_ps[:])

            # ---- adtab1 from own x slice (per-core asymmetry lives in
            # the xown input, so the SPMD program stays static) ----
            with tc.tile_pool(name="xop", bufs=1) as xop, \
                 tc.tile_pool(name="adps0", bufs=2, space="PSUM") as adps0:
                xown_s = xop.tile([P, NPCPAD], bf16)
                nc.sync.dma_start(out=xown_s[:], in_=xown_d[:])
                for t in range(NT):
                    adn_ps = adps0.tile([P, 2], f32)
                    nc.tensor.matmul(out=adn_ps[:],
                                     lhsT=xown_s[:, t * P:(t + 1) * P],
                                     rhs=u1bf[:, 2:4], start=True, stop=True)
                    nc.vector.tensor_copy(out=adtab1[:, t * 2:t * 2 + 2],
                                          in_=adn_ps[:])

            # ---- node phase 1: full-table sweep from replicated x.
            # Per core block: 6 loads of 8 tiles + 1 single; writes are
            # 8-tile interleaved (rows g*1024 + p*8 + i) so 8 tiles go out
            # in one DMA at 4KB/partition. Loads on the ACT HWDGE, writes
            # on SP, copies alternate vector/scalar. ----
            with tc.tile_pool(name="xnp", bufs=3) as xnp, \
                 tc.tile_pool(name="n1", bufs=3) as n1, \
                 tc.tile_pool(name="n1ps", bufs=4, space="PSUM") as n1ps:
                for c in range(NCORES):
                    base = c * NPCPAD
                    for l in range(7):
                        nt8 = 8 if l < 6 else 1
                        xt = xnp.tile([P, 8 * P], bf16, name="xt8")
                        nc.sync.dma_start(
                            out=xt[:, 0:nt8 * P],
                            in_=xfull_d[:, base + l * 8 * P:
                                        base + (l * 8 + nt8) * P])
                        rec8t = n1.tile([P, 8 * RECW], bf16, name="rec8t")
                        for i in range(nt8):
                            ps1 = n1ps.tile([P, 132], f32, name="ps1")
                            nc.tensor.matmul(
                                out=ps1[:], lhsT=xt[:, i * P:(i + 1) * P],
                                rhs=rhs1[:], start=True, stop=True)
                            rv = rec8t[:, i * RECW:(i + 1) * RECW]
                            if i % 2 == 0:
                                nc.vector.tensor_copy(
                                    out=rv.rearrange(
                                        "p (h q) -> p h q",
                                        q=P)[:, :, 4:4 + HID],
                                    in_=ps1[:, 0:128].rearrange(
                                        "p (h c) -> p h c", c=HID))
                                nc.vector.tensor_copy(out=rv[:, 0:4],
                                                      in_=ps1[:, 128:132])
                            else:
                                nc.scalar.activation(
                                    rv.rearrange(
                                        "p (h q) -> p h q",
                                        q=P)[:, :, 4:4 + HID],
                                    ps1[:, 0:128].rearrange(
                                        "p (h c) -> p h c", c=HID),
                                    AF.Copy)
                                nc.scalar.activation(rv[:, 0:4],
                                                     ps1[:, 128:132],
                                                     AF.Copy)
                        if nt8 == 8:
                            nc.sync.dma_start(
                                out=rec1_full[
                                    base + l * 1024:base + (l + 1) * 1024,
                                    :].rearrange("(p i) r -> p i r", i=8),
                                in_=rec8t[:].rearrange(
                                    "p (i r) -> p i r", i=8))
                        else:
                            nc.sync.dma_start(
                                out=rec1_full[base + 6144:base + 6272, :],
                                in_=rec8t[:, 0:RECW])

            # ---- edge phase (both layers share one pool scope so the
            # scheduler can overlap L2 batch prep with the AllGather) ----
            def edge_phase(layer, full_tab, adtab, normalize,
                           ep, epa, np_, eps, adps_p, nps):
                ngrp = 2                       # heads (L1) / mu-std groups (L2)
                cw = HID if layer == 1 else Z  # payload cols per group
                ww = ngrp * (cw + 1)           # 130 / 66
                recw = RECW if layer == 1 else RECW2
                qstride = P if layer == 1 else 64
                esrc_l = esrc_d if layer == 1 else esrc2_d
                viewA = full_tab[0:HALFROWS, :]
                viewB = full_tab[HALFROWS:2 * HALFROWS, :]
                if True:
                    state = {"a0": None, "w": None, "b0": 0, "q": 0}

                    def emit_batch(b0):
                        bn = min(NB, nblk - b0)
                        esrc_t = ep.tile([P, NB * 8], i16, name="esrc")
                        dslot = ep.tile([P, NB], bf16, name="dslot")
                        pmk = ep.tile([P, NB], bf16, name="pmk")
                        nc.sync.dma_start(out=esrc_t[:, 0:bn * 8],
                                          in_=esrc_l[:, b0 * 8:(b0 + bn) * 8])
                        nc.sync.dma_start(out=dslot[:, 0:bn],
                                          in_=eslot_d[:, b0:b0 + bn])
                        nc.sync.dma_start(out=pmk[:, 0:bn],
                                          in_=pmask_d[:, b0:b0 + bn])
                        eslotT = ep.tile([P, NB * P], mybir.dt.uint8,
                                         name="eslT")
                        nc.sync.dma_start(
                            out=eslotT[:, 0:bn * P],
                            in_=eslotf_d[0:1, b0 * P:(b0 + bn) * P].to_broadcast(
                                [P, bn * P]))
                        rec_g = ep.tile([P, NB * RECW], bf16,
                                        name="rec_g")
                        # gather: per same-half run of blocks, equal splits
                        # (never a tiny tail call), RR over the 4 SWDGE
                        # queues (each runs on its own Q7 core pair)
                        r0 = 0
                        while r0 < bn:
                            hf = half_flags[b0 + r0]
                            rg = r0 + 1
                            while rg < bn and half_flags[b0 + rg] == hf:
                                rg += 1
                            glen = rg - r0
                            nparts = -(-glen // GMAXB)
                            plen = -(-glen // nparts)
                            r1 = min(r0 + plen, rg)
                            nrun = (r1 - r0) * P
                            nc.gpsimd.dma_gather(
                                out_ap=rec_g[:, r0 * recw:r1 * recw].rearrange(
                                    "p (g e) -> p g e", e=recw),
                                in_ap=(viewB if hf else viewA),
                                idxs_ap=esrc_t[:, r0 * 8:r1 * 8],
                                num_idxs=nrun, num_idxs_reg=nrun,
                                elem_size=recw,
                                queue_num=state["q"])
                            state["q"] = (state["q"] + 1) % 4
                            r0 = r1
                        # transposed one-hot (dst-slot x edge): per-partition
                        # scalar compare -> DVE fast path
                        a0T = epa.tile([P, NB * P], bf16, name="a0T")
                        nc.vector.tensor_scalar(
                            out=a0T[:, 0:bn * P],
                            in0=eslotT[:, 0:bn * P],
                            scalar1=iota_col[:, 0:1], scalar2=None,
                            op0=OP.is_equal)
                        ad_ps = adps_p.tile([P, NB * 2], f32, name="adps")
                        for o in range(bn):
                            t = int(tile_of_block[b0 + o])
                            nc.tensor.matmul(
                                out=ad_ps[:, o * 2:(o + 1) * 2],
                                lhsT=a0T[:, o * P:(o + 1) * P],
                                rhs=adtab[:, t * 2:t * 2 + 2],
                                start=True, stop=True)
                        # t = as + ad ; u = max(.2t, t) ; pex = exp(u)
                        tt = ep.tile([P, NB * 2], bf16, name="tt")
                        nc.vector.tensor_tensor(
                            out=tt[:, 0:bn * 2].rearrange("p (b h) -> p b h", h=2),
                            in0=rec_g[:, 0:bn * recw].rearrange(
                                "p (b r) -> p b r", r=recw)[:, :, 0:2],
                            in1=ad_ps[:, 0:bn * 2].rearrange(
                                "p (b h) -> p b h", h=2),
                            op=OP.add)
                        uu = ep.tile([P, NB * 2], bf16, name="uu")
                        nc.vector.scalar_tensor_tensor(
                            out=uu[:, 0:bn * 2], in0=tt[:, 0:bn * 2],
                            scalar=NEG, in1=tt[:, 0:bn * 2],
                            op0=OP.mult, op1=OP.max)
                        # pad edges: add -1e30 so exp() -> 0
                        uum = ep.tile([P, NB * 2], bf16, name="uum")
                        nc.vector.tensor_tensor(
                            out=uum[:, 0:bn * 2].rearrange(
                                "p (b h) -> p b h", h=2),
                            in0=uu[:, 0:bn * 2].rearrange(
                                "p (b h) -> p b h", h=2),
                            in1=pmk[:, 0:bn][:, :, None].to_broadcast(
                                [P, bn, 2]),
                            op=OP.add)
                        pex = ep.tile([P, NB * 2], bf16, name="pex")
                        nc.scalar.activation(pex[:, 0:bn * 2], uum[:, 0:bn * 2],
                                             AF.Exp)
                        # A0 one-hot
                        a0 = epa.tile([P, NB * P], bf16, name="a0_")
                        nc.vector.tensor_tensor(
                            out=a0[:, 0:bn * P].rearrange("p (b r) -> p b r", r=P),
                            in0=dslot[:, 0:bn][:, :, None].to_broadcast([P, bn, P]),
                            in1=iota_bf[:][:, None, :].to_broadcast([P, bn, P]),
                            op=OP.is_equal)
                        # w build: per group [payload*pex | pex]
                        w = epa.tile([P, NB * (2 * (HID + 1))], bf16, name="w")
                        rec4 = rec_g[:, 0:bn * recw].rearrange(
                            "p (b g q) -> p b g q", g=2, q=qstride)[
                            :, :, :, 4:4 + cw]
                        wv = w[:, 0:bn * ww].rearrange(
                            "p (b g c) -> p b g c", g=2, c=cw + 1)
                        pex3 = pex[:, 0:bn * 2].rearrange("p (b g) -> p b g", g=2)
                        nc.vector.tensor_tensor(
                            out=wv[:, :, :, 0:cw], in0=rec4,
                            in1=pex3[:, :, :, None].to_broadcast(
                                [P, bn, 2, cw]),
                            op=OP.mult)
                        nc.scalar.activation(
                            wv[:, :, :, cw:cw + 1],
                            pex3[:, :, :, None], AF.Copy)
                        state["a0"], state["w"], state["b0"] = a0, w, b0

                    B = 0
                    for T in range(NT):
                        ps = eps.tile([P, 2 * (HID + 1)], f32, name="acc")
                        psv = ps[:, 0:ww]
                        kb = int(schedule[T].sum())
                        for j in range(kb):
                            if state["a0"] is None or B >= state["b0"] + NB:
                                emit_batch(B)
                            o = B - state["b0"]
                            nc.tensor.matmul(
                                out=psv,
                                lhsT=state["a0"][:, o * P:(o + 1) * P],
                                rhs=state["w"][:, o * ww:(o + 1) * ww],
                                start=(j == 0), stop=(j == kb - 1))
                            B += 1
                        normalize(psv, T, np_, nps)

            # ---- normalize callbacks ----
            def norm1(ps, T, np_, nps):
                ps3 = ps.rearrange("p (h c) -> p h c", c=HID + 1)
                se = np_.tile([P, 2], f32, name="se1")
                nc.scalar.activation(
                    se[:].rearrange("p (h o) -> p h o", o=1),
                    ps3[:, :, HID:HID + 1], AF.Copy, bias=1e-30)
                rs = np_.tile([P, 2], f32, name="rs1")
                nc.vector.reciprocal(rs[:], se[:])
                h_f = np_.tile([P, F_IN], f32, name="h_f")
                hv = h_f[:].rearrange("p (h c) -> p h c", c=HID)
                for hh in range(2):
                    nc.scalar.activation(
                        hv[:, hh], ps3[:, hh, 0:HID], AF.Copy,
                        scale=rs[:, hh:hh + 1])
                nc.vector.tensor_tensor(out=h_f[:], in0=h_f[:], in1=b1rep_s,
                                        op=OP.add)
                hr_f = np_.tile([P, F_IN], f32, name="hr_f")
                nc.scalar.activation(hr_f[:], h_f[:], AF.Relu)
                hT_ps = nps.tile([P, P], f32, name="hTps")
                nc.tensor.transpose(out=hT_ps[:], in_=hr_f[:], identity=ident[:])
                hT_s = np_.tile([P, P], bf16, name="hTs")
                nc.scalar.activation(hT_s[:], hT_ps[:], AF.Copy)
                # [alphas2(4) | xpmu(32) | xpstd(32)] in one matmul
                ps2 = nps.tile([P, 68], f32, name="ps2")
                nc.tensor.matmul(out=ps2[:], lhsT=hT_s[:], rhs=rhscat[:],
                                 start=True, stop=True)
                rec2_t = np_.tile([P, RECW2], bf16, name="rec2t")
                nc.scalar.activation(rec2_t[:, 0:36], ps2[:, 0:36], AF.Copy)
                nc.scalar.activation(rec2_t[:, 68:100], ps2[:, 36:68], AF.Copy)
                nc.vector.tensor_copy(out=adtab2[:, T * 2:T * 2 + 2],
                                      in_=ps2[:, 2:4])
                nc.sync.dma_start(out=rec2_slice[T * P:(T + 1) * P, :],
                                  in_=rec2_t[:])

            def norm2(ps, T, np_, nps):
                ps3 = ps.rearrange("p (g c) -> p g c", c=Z + 1)
                se = np_.tile([P, 2], f32, name="se2")
                nc.scalar.activation(
                    se[:].rearrange("p (g o) -> p g o", o=1),
                    ps3[:, :, Z:Z + 1], AF.Copy, bias=1e-30)
                rs = np_.tile([P, 2], f32, name="rs2")
                nc.vector.reciprocal(rs[:], se[:])
                o_s = np_.tile([P, 2 * Z], f32, name="outs")
                ov = o_s[:].rearrange("p (g c) -> p g c", c=Z)
                for gg in range(2):
                    nc.scalar.activation(
                        ov[:, gg], ps3[:, gg, 0:Z], AF.Copy,
                        scale=rs[:, gg:gg + 1])
                nc.vector.tensor_tensor(out=o_s[:], in0=o_s[:], in1=b2rep_s,
                                        op=OP.add)
                rows = min(P, NPC - T * P)
                nc.sync.dma_start(
                    out=musd_out[T * P:T * P + rows, :],
                    in_=o_s[0:rows, :])

            with tc.tile_pool(name="ep", bufs=4) as ep, \
                 tc.tile_pool(name="epa", bufs=4) as epa, \
                 tc.tile_pool(name="npx", bufs=3) as np_, \
                 tc.tile_pool(name="eps", bufs=2, space="PSUM") as eps, \
                 tc.tile_pool(name="adps", bufs=2, space="PSUM") as adps_p, \
                 tc.tile_pool(name="nxps", bufs=2, space="PSUM") as nps:
                edge_phase(1, rec1_full, adtab1, norm1,
                           ep, epa, np_, eps, adps_p, nps)
                _collective("AllGather", OP.bypass,
                            [list(range(NCORES))],
                            [rec2_slice[:]], [rec2_full[:]])
                edge_phase(2, rec2_full, adtab2, norm2,
                           ep, epa, np_, eps, adps_p, nps)

    nc.compile()
    return nc


# ---------------- runner ----------------
_CACHE = {}


def _get_runner(schedule, half_flags):
    key = tuple(schedule.reshape(-1).tolist())
    if key not in _CACHE:
        nc = _build_nc(schedule, half_flags)
        _CACHE[key] = (nc, {})
    return _CACHE[key]


def run_on_hw(inputs_per_core, schedule, half_flags):
    import jax
    from concourse import bass2jax
    nc, captured = _get_runner(schedule, half_flags)
    orig_jit = jax.jit

    def cap_jit(fun, **kw):
        j = orig_jit(fun, **kw)
        captured['fn'] = j
        return j
    jax.jit = cap_jit
    try:
        results = bass2jax.run_bass_via_pjrt(nc, inputs_per_core, n_cores=NCORES)
    finally:
        jax.jit = orig_jit
    return results, captured.get('fn'), nc


def make_inputs_per_core(features, edges, wp):
    schedule, half_flags, per_core = _prep_edges(np.asarray(edges))
    nblk = int(schedule.sum())
    feats = np.asarray(features, np.float32)
    xfull = np.zeros((P, NPCPAD * NCORES), BF)
    for c in range(NCORES):
        xfull[:, c * NPCPAD:c * NPCPAD + NPC] = feats[c * NPC:(c + 1) * NPC].T
    ins = []
    for c in range(NCORES):
        xall = np.concatenate(
            [xfull, xfull[:, c * NPCPAD:(c + 1) * NPCPAD],
             per_core[c]["eslot"], per_core[c]["pmask"], wp["wbf"]], axis=1)
        ins.append({"xall": np.ascontiguousarray(xall),
                    "wblob": wp["wblob"],
                    "esall": per_core[c]["esall"],
                    "eslotf": per_core[c]["eslotf"]})
    return schedule, half_flags, ins


def kernel(features, edges, W1, a_src1, a_dst1, b1, W_mu, a_src_mu, a_dst_mu,
           b_mu, W_std, a_src_std, a_dst_std, b_std):
    wp = _prep_weights(np.asarray(W1), np.asarray(a_src1), np.asarray(a_dst1),
                       np.asarray(b1), np.asarray(W_mu), np.asarray(a_src_mu),
                       np.asarray(a_dst_mu), np.asarray(b_mu), np.asarray(W_std),
                       np.asarray(a_src_std), np.asarray(a_dst_std),
                       np.asarray(b_std))
    schedule, half_flags, ins = make_inputs_per_core(features, edges, wp)
    results, _, _ = run_on_hw(ins, schedule, half_flags)
    musd = np.concatenate([results[c]["musd_out"] for c in range(NCORES)],
                          axis=0)
    return (np.ascontiguousarray(musd[:, 0:Z]),
            np.ascontiguousarray(musd[:, Z:2 * Z]))
